# revision 21
# baseline (speedup 1.0000x reference)
"""VQ codebook kernel for Trainium2 (8 NeuronCores, data-parallel over batch).

dist[b,p] = sum_{t,d} mask[b,t] * (x[b,t,d] - proto[p,t,d])^2
          = term_x[b] - 2*cross[b,p] + term_p[b,p]
indices = argmin_p dist ; output_seq = protos[indices]

Layout trick: ship x as [t, d, b] and protos as [t, d, p] (t padded to 384).
Then the mask multiply is a free-dim broadcast on DVE, and the GEMM is 32
per-d matmuls with t on partitions, accumulated in PSUM.  term_p folds in as
3 extra matmuls (lhsT=maskT slice, rhs=psq[t,p]), term_x via matmuls of
mask * sum_d x^2 against a constant ones column.  Gather is an indirect DMA
from DRAM protos + store.
"""

import numpy as np

import concourse.bass as bass
import concourse.bacc as bacc
import concourse.mybir as mybir
from concourse.tile import TileContext
from concourse.bass_utils import run_bass_kernel_spmd

F32 = mybir.dt.float32
I32 = mybir.dt.int32
ALU = mybir.AluOpType
AX = mybir.AxisListType
AF = mybir.ActivationFunctionType

import os

B, P, T, D = 2048, 256, 365, 32
NCORES = 8
BS = B // NCORES        # 256 rows per core
BBLK = BS // 128        # 2 batch blocks of 128
TP = 384                # t padded to 3*128
TT = TP // 128          # 3 t-tiles
DH = int(os.environ.get("VQ_DH", "16"))   # d chunk size
NDH = D // DH
KTD = T * D             # 11680
GCH = int(os.environ.get("VQ_GCH", "4"))  # gather column chunks
GW = KTD // GCH
XBUFS = int(os.environ.get("VQ_XBUFS", "2"))
GBUFS = int(os.environ.get("VQ_GBUFS", "2"))

BIG = 4096.0            # argmin sentinel offset


def build_bass():
    nc = bacc.Bacc(None, target_bir_lowering=False)

    xT = nc.dram_tensor("xT", [TP, D, BS], F32, kind="ExternalInput")
    pT = nc.dram_tensor("pT", [TP, D, P], F32, kind="ExternalInput")
    mT = nc.dram_tensor("mT", [TP, BS], F32, kind="ExternalInput")
    pnat = nc.dram_tensor("pnat", [P, KTD], F32, kind="ExternalInput")
    onesc = nc.dram_tensor("onesc", [128, 1], F32, kind="ExternalInput")  # +1.0

    dist_o = nc.dram_tensor("dist", [BS, P], F32, kind="ExternalOutput")
    idx_o = nc.dram_tensor("idxo", [BS, 1], I32, kind="ExternalOutput")
    oseq_o = nc.dram_tensor("oseq", [BS, KTD], F32, kind="ExternalOutput")

    with TileContext(nc) as tc:
        with (
            tc.tile_pool(name="const", bufs=1) as cpool,
            tc.tile_pool(name="mask", bufs=TT) as mpool,
            tc.tile_pool(name="xt", bufs=XBUFS) as xpool,
            tc.tile_pool(name="pt", bufs=XBUFS) as ppool,
            tc.tile_pool(name="mx", bufs=XBUFS) as mxpool,
            tc.tile_pool(name="sq", bufs=2) as sqpool,
            tc.tile_pool(name="small", bufs=2) as spool,
            tc.tile_pool(name="gat", bufs=GBUFS) as gpool,
            tc.tile_pool(name="psum", bufs=1, space="PSUM") as pspool,
        ):
            # ---- constants ----
            ones = cpool.tile([128, 1], F32, tag="ones")
            nc.sync.dma_start(out=ones[:], in_=onesc[:])
            iotam = cpool.tile([128, P], F32, tag="iotam")
            nc.gpsimd.iota(
                iotam[:], pattern=[[1, P]], base=int(-BIG),
                channel_multiplier=0, allow_small_or_imprecise_dtypes=True,
            )
            # advance DVE's observed Pool clock past the iota so the argmin
            # STT later needs only its same-engine wait (STT has 1 wait slot)
            idum = cpool.tile([128, 1], F32, tag="idum")
            nc.vector.tensor_copy(out=idum[:], in_=iotam[:, :1])

            # ---- mask tiles [128 t, BS b] ----
            mtiles = []
            for tt in range(TT):
                mtile = mpool.tile([128, BS], F32, tag="mask")
                nc.sync.dma_start(out=mtile[:], in_=mT[tt * 128:(tt + 1) * 128, :])
                # wait-absorber: advance DVE's clock past this DMA so later
                # DVE consumers carry at most one cross-engine wait
                # (walrus STT/TT encodings have a single wait slot).
                mdum = spool.tile([128, 1], F32, tag="mdum")
                nc.vector.tensor_copy(out=mdum[:], in_=mtile[:, :1])
                mtiles.append(mtile)

            # ---- PSUM accumulators ----
            cross_ps = [
                pspool.tile([128, P], F32, tag=f"cross{b}", name=f"cross_ps{b}")
                for b in range(BBLK)
            ]
            tp_ps = [
                pspool.tile([128, P], F32, tag=f"tp{b}", name=f"tp_ps{b}")
                for b in range(BBLK)
            ]
            tx_ps = [
                pspool.tile([128, 1], F32, tag=f"tx{b}", name=f"tx_ps{b}")
                for b in range(BBLK)
            ]

            first_mm = [True] * BBLK
            first_tx = [True] * BBLK
            # ---- main K loop ----
            for tt in range(TT):
                psq_h = []
                for dh in range(NDH):
                    d0 = dh * DH
                    xt = xpool.tile([128, DH, BS], F32, tag="xt")
                    nc.sync.dma_start(
                        out=xt[:], in_=xT[tt * 128:(tt + 1) * 128, d0:d0 + DH, :])
                    pt = ppool.tile([128, DH, P], F32, tag="pt")
                    nc.sync.dma_start(
                        out=pt[:], in_=pT[tt * 128:(tt + 1) * 128, d0:d0 + DH, :])

                    # mx = mask * x   (mask free-broadcast over d; the -2 is
                    # applied at distance assembly)
                    mx = mxpool.tile([128, DH, BS], F32, tag="mx")
                    mb = mtiles[tt][:].rearrange("p (o b) -> p o b", o=1).to_broadcast(
                        [128, DH, BS])
                    nc.vector.tensor_tensor(
                        out=mx[:], in0=xt[:], in1=mb, op=ALU.mult)

                    # squares on ACT engine
                    sqx = sqpool.tile([128, DH, BS], F32, tag="sq")
                    nc.scalar.activation(out=sqx[:], in_=xt[:], func=AF.Square)
                    # xsq_h[t,b] = sum_d x^2 ; mxs_h = mask * xsq_h
                    xsq = spool.tile([128, BS], F32, tag="xsq")
                    nc.vector.tensor_reduce(
                        out=xsq[:], in_=sqx[:].rearrange("p d b -> p b d"),
                        axis=AX.X, op=ALU.add)
                    mxs = spool.tile([128, BS], F32, tag="mxs")
                    nc.vector.tensor_tensor(
                        out=mxs[:], in0=xsq[:], in1=mtiles[tt][:], op=ALU.mult)

                    sqp = sqpool.tile([128, DH, P], F32, tag="sq")
                    nc.scalar.activation(out=sqp[:], in_=pt[:], func=AF.Square)
                    ph = spool.tile([128, P], F32, tag="psqh")
                    nc.vector.tensor_reduce(
                        out=ph[:], in_=sqp[:].rearrange("p d q -> p q d"),
                        axis=AX.X, op=ALU.add)
                    psq_h.append(ph)

                    # term_x: tx_ps[b] += mxs[:, blk].T @ ones
                    for bb in range(BBLK):
                        nc.tensor.matmul(
                            out=tx_ps[bb][:],
                            lhsT=mxs[:, bb * 128:(bb + 1) * 128],
                            rhs=ones[:],
                            start=first_tx[bb], stop=(tt == TT - 1 and dh == NDH - 1),
                        )
                        first_tx[bb] = False

                    # cross GEMM: cross_ps[bb] += sum_d mx[:,d,blk].T @ pt[:,d,:]
                    # The very last d-matmul batch is deferred until after the
                    # final term_p matmul, so the PE clock at cross-stop covers
                    # everything (keeps later STT waits within one slot).
                    if tt == TT - 1 and dh == NDH - 1:
                        deferred = (mx, pt)
                    else:
                        for bb in range(BBLK):
                            for d in range(DH):
                                nc.tensor.matmul(
                                    out=cross_ps[bb][:],
                                    lhsT=mx[:, d, bb * 128:(bb + 1) * 128],
                                    rhs=pt[:, d, :],
                                    start=first_mm[bb], stop=False,
                                )
                                first_mm[bb] = False

                # psq_tt = psq_h0 + psq_h1 ; term_p matmuls
                psq = spool.tile([128, P], F32, tag="psq")
                nc.vector.tensor_tensor(
                    out=psq[:], in0=psq_h[0][:], in1=psq_h[1][:], op=ALU.add)
                for bb in range(BBLK):
                    nc.tensor.matmul(
                        out=tp_ps[bb][:],
                        lhsT=mtiles[tt][:, bb * 128:(bb + 1) * 128],
                        rhs=psq[:],
                        start=(tt == 0), stop=(tt == TT - 1),
                    )

            # deferred final cross matmuls (after all other PE work)
            mx, pt = deferred
            for bb in range(BBLK):
                for d in range(DH):
                    nc.tensor.matmul(
                        out=cross_ps[bb][:],
                        lhsT=mx[:, d, bb * 128:(bb + 1) * 128],
                        rhs=pt[:, d, :],
                        start=False, stop=(d == DH - 1),
                    )

            # ---- distances, argmin, gather per batch block ----
            for bb in range(BBLK):
                # absorb the PE wait (cross stop covers tp/tx stops too) into a
                # 2-wait-capable copy, so the assembly STTs below need only
                # their same-engine wait
                pdum = spool.tile([128, 1], F32, tag="pdum")
                nc.vector.tensor_copy(out=pdum[:], in_=cross_ps[bb][:, :1])

                # tx_ps holds term_x = sum_td m x^2
                txc = spool.tile([128, 1], F32, tag="txc")
                nc.vector.tensor_copy(out=txc[:], in_=tx_ps[bb][:])

                # dist = -2*cross + term_x + term_p
                dist_sb = spool.tile([128, P], F32, tag="dist_sb")
                nc.vector.tensor_scalar(
                    out=dist_sb[:], in0=cross_ps[bb][:],
                    scalar1=-2.0, scalar2=txc[:, :1],
                    op0=ALU.mult, op1=ALU.add)
                nc.vector.tensor_tensor(
                    out=dist_sb[:], in0=dist_sb[:], in1=tp_ps[bb][:], op=ALU.add)
                nc.sync.dma_start(
                    out=dist_o[bb * 128:(bb + 1) * 128, :], in_=dist_sb[:])

                minv = spool.tile([128, 1], F32, tag="minv")
                nc.vector.tensor_reduce(
                    out=minv[:], in_=dist_sb[:], axis=AX.X, op=ALU.min)
                cand = spool.tile([128, P], F32, tag="cand")
                nc.vector.scalar_tensor_tensor(
                    out=cand[:], in0=dist_sb[:], scalar=minv[:, :1], in1=iotam[:],
                    op0=ALU.is_equal, op1=ALU.mult)
                idxm = spool.tile([128, 1], F32, tag="idxm")
                nc.vector.tensor_reduce(
                    out=idxm[:], in_=cand[:], axis=AX.X, op=ALU.min)
                idxf = spool.tile([128, 1], F32, tag="idxf")
                nc.vector.tensor_scalar_add(out=idxf[:], in0=idxm[:], scalar1=BIG)
                idxi = spool.tile([128, 1], I32, tag="idxi")
                nc.vector.tensor_copy(out=idxi[:], in_=idxf[:])
                nc.sync.dma_start(
                    out=idx_o[bb * 128:(bb + 1) * 128, :], in_=idxi[:])

                for cc in range(GCH):
                    gt = gpool.tile([128, GW], F32, tag="gt")
                    nc.gpsimd.indirect_dma_start(
                        out=gt[:], out_offset=None,
                        in_=pnat[:],
                        in_offset=bass.IndirectOffsetOnAxis(ap=idxi[:, :1], axis=0),
                        element_offset=cc * GW,
                    )
                    nc.sync.dma_start(
                        out=oseq_o[bb * 128:(bb + 1) * 128, cc * GW:(cc + 1) * GW],
                        in_=gt[:])
    return nc


_NC_CACHE = None


def _get_nc():
    global _NC_CACHE
    if _NC_CACHE is None:
        nc = build_bass()
        nc.finalize()  # run the bacc passes (wait splitting, reg alloc)
        _NC_CACHE = nc
    return _NC_CACHE


def kernel(**inputs) -> tuple:
    x = np.ascontiguousarray(np.asarray(inputs["input_seq"], dtype=np.float32))
    label = np.asarray(inputs["label"])
    mask = np.ascontiguousarray(np.asarray(inputs["mask"], dtype=np.float32))
    protos = np.ascontiguousarray(np.asarray(inputs["prototypes"], dtype=np.float32))

    # host-side sharding + layout (no arithmetic)
    xTs = np.zeros((NCORES, TP, D, BS), np.float32)
    xTs[:, :T] = x.reshape(NCORES, BS, T, D).transpose(0, 2, 3, 1)
    mTs = np.zeros((NCORES, TP, BS), np.float32)
    mTs[:, :T] = mask.reshape(NCORES, BS, T).transpose(0, 2, 1)
    pT = np.zeros((TP, D, P), np.float32)
    pT[:T] = protos.transpose(1, 2, 0)
    pnat = np.ascontiguousarray(protos.reshape(P, KTD))
    onesc = np.full((128, 1), 1.0, np.float32)

    in_maps = [
        {
            "xT": xTs[i],
            "pT": pT,
            "mT": mTs[i],
            "pnat": pnat,
            "onesc": onesc,
        }
        for i in range(NCORES)
    ]

    nc = _get_nc()
    res = run_bass_kernel_spmd(nc, in_maps, core_ids=list(range(NCORES)))
    rs = res.results

    dist = np.concatenate([r["dist"] for r in rs], axis=0)
    idx = np.concatenate([r["idxo"][:, 0] for r in rs], axis=0).astype(np.int32)
    oseq = np.concatenate([r["oseq"] for r in rs], axis=0).reshape(B, T, D)

    return (
        oseq,
        np.asarray(inputs["input_seq"]),
        dist,
        idx,
        label,
        np.asarray(inputs["mask"]),
    )
